# revision 33
# baseline (speedup 1.0000x reference)
"""Trainium2 Bass kernel for the directional min-variance filter (Kuwahara-style).

Algorithm (per image, fp32):
  For each of 8 directions d (rays of 8 pixels from each pixel):
    x1_d = directional sum of x, y2_d = directional sum of x^2
    metric m'_d = x1_d^2/8 - y2_d          (= y1 - y2; maximize m' == minimize var)
  out = x1_{argmax m'} / 8   with first-index-wins tie semantics (matches argmin).

Layout ("all free-dim"): the 1024x1024 image is split into 128 blocks of
64 rows x 128 cols; partition p = cb*16 + rc owns block (rc, cb) and stores
it row-major with a 7-pixel halo on every side: 78 rows x 142 cols.  Every
directional shift is then a pure free-dim offset, so all sums run on the
vector engine with plain 2-operand adds (log2 doubling: 3 adds per 8-sum).
Forward/backward direction pairs share sums: the backward ray sum at (y,x)
equals the forward sum at (y,x) - 7*u, read as an offset view.

The per-row-slab pipeline (8 rows at a time) keeps forward sums and metrics
in 16-row double-block buffers: each slab computes only its own 8 new rows;
backward-direction views read the previous slab's block (contiguous view for
odd slabs, a 2-way row split for even slabs).

8 NeuronCores run pure data-parallel over the batch of 8 images.
"""

import numpy as np

import concourse.bass as bass
import concourse.bacc as bacc
import concourse.tile as tile
from concourse import mybir
from concourse.bass_utils import run_bass_kernel_spmd

F32 = mybir.dt.float32

H = W = 1024
A = 128         # rows per block
B = 64          # cols per block
NRC = 8         # row-chunks  (rc)
NCB = 16        # col-blocks  (cb)
PITCH = B + 14  # 78
XR = A + 14     # 142 stored rows
S = 16          # slab rows
NS = A // S     # 8 slabs

PAD = 16
SZ_X = XR * PITCH              # 11076
SZ_XSQ = (S + 14) * PITCH      # 30 rows (23 normally; 30 for slab 0)
SZ_S1 = (S + 6) * PITCH
SZ_S2 = (S + 4) * PITCH
SZ_F = 2 * S * PITCH           # two S-row blocks
SZ_Y2 = S * PITCH
SZ_O = S * PITCH

OFF_X = 0
OFF_XSQ = OFF_X + SZ_X + PAD                    # 2 rotating slots
OFF_S1 = OFF_XSQ + 2 * (SZ_XSQ + PAD)           # plane 0 (x) + plane 1 (x^2)
OFF_S1B = OFF_S1 + SZ_S1 + PAD
OFF_S2 = OFF_S1B + SZ_S1 + PAD
OFF_S2B = OFF_S2 + SZ_S2 + PAD
OFF_F = OFF_S2B + SZ_S2 + PAD                   # 4 x 2S-row (x1 per fwd dir)
OFF_Y2 = OFF_F + 4 * (SZ_F + PAD)               # 2 rotating slots
OFF_M = OFF_Y2 + 2 * (SZ_Y2 + PAD)              # 4 x 2S-row (metric per dir)
OFF_BM = OFF_M + 4 * (SZ_F + PAD)               # 2 rotating slots
OFF_BX = OFF_BM + 2 * (SZ_O + PAD)              # 2 rotating slots
OFF_OUT = OFF_BX + 2 * (SZ_O + PAD)             # 2 rotating slots
TOTAL = OFF_OUT + 2 * (SZ_O + PAD) + PITCH

XWAVES = ((0, 34), (34, XR))  # input DMA row waves

# forward dirs: name -> (uy, ux, slot index, F col range (x0, x1))
FWD = {
    "a": (0, 1, 0, (-7, B)),
    "b": (1, 0, 1, (0, B)),
    "c": (1, 1, 2, (-7, B)),
    "e": (1, -1, 3, (0, B + 7)),
}
# chain in reference dir order: (fwd-name, dy, dx): the d-th direction's
# metric/payload = fwd buffer at row offset dy, col offset dx
CHAIN = [
    ("c", -7, -7),  # d0 (-1,-1)
    ("b", -7, 0),   # d1 (-1, 0)
    ("e", -7, 7),   # d2 (-1, 1)
    ("a", 0, -7),   # d3 (0,-1)
    ("a", 0, 0),    # d4 (0, 1)
    ("e", 0, 0),    # d5 (1,-1)
    ("b", 0, 0),    # d6 (1, 0)
    ("c", 0, 0),    # d7 (1, 1)
]


def _build():
    nc = bacc.Bacc("TRN2", target_bir_lowering=False)
    x_t = nc.declare_dram_parameter("x", [H, W], F32, isOutput=False)
    y_t = nc.declare_dram_parameter("y", [H, W], F32, isOutput=True)
    x_ap = x_t[:]
    y_ap = y_t[:]

    with tile.TileContext(nc) as tc:
        with tc.tile_pool(name="main", bufs=1) as pool:
            big = pool.tile([128, TOTAL], F32)
            lt8 = pool.tile([128, S * PITCH + PAD], mybir.dt.uint8)

            def view(off, r0, r1, c0, c1, p0=0, p1=128):
                # [p1-p0, r1-r0, c1-c0] view of a pitch-PITCH buffer at `off`
                start = off + r0 * PITCH + c0
                ln = (r1 - r0) * PITCH
                return big[p0:p1, start:start + ln].rearrange(
                    "p (r c) -> p r c", c=PITCH)[:, :, 0:c1 - c0]

            def xv(y0, y1, x0, x1):
                # X view in image coords (origin row -7, col -7)
                return view(OFF_X, y0 + 7, y1 + 7, x0 + 7, x1 + 7)

            # ------- input load (row-waves of clipped DMAs) -------
            # wave 1 covers the first X rows so slab 0 can start while wave 2
            # is in flight.  Only the halo strips (left unwritten on edge
            # partitions) need zeroing; the interior is DMA-overwritten.
            nc.vector.memset(view(OFF_X, 0, 7, 0, PITCH), 0.0)
            nc.vector.memset(view(OFF_X, XR - 7, XR, 0, PITCH), 0.0)
            nc.vector.memset(view(OFF_X, 7, XR - 7, 0, 7), 0.0)
            nc.vector.memset(view(OFF_X, 7, XR - 7, PITCH - 7, PITCH), 0.0)
            for xr0, xr1 in XWAVES:
                for cb in range(NCB):
                    c0s = cb * B - 7
                    wn, cd = PITCH, 0
                    if cb == 0:
                        c0s, wn, cd = 0, PITCH - 7, 7
                    elif cb == NCB - 1:
                        wn = PITCH - 7
                    p0 = cb * NRC
                    # rc == 0 (top edge: img rows [0,71) -> X rows [7,78))
                    r0, r1 = max(xr0, 7), xr1
                    nc.sync.dma_start(
                        out=view(OFF_X, r0, r1, cd, cd + wn, p0, p0 + 1),
                        in_=bass.AP(tensor=x_ap.tensor,
                                    offset=x_ap.offset + (r0 - 7) * W + c0s,
                                    ap=[[0, 1], [W, r1 - r0], [1, wn]]))
                    # rc in [1, 15): X rows [0,78) <- img rows rc*A-7 ...
                    nc.sync.dma_start(
                        out=view(OFF_X, xr0, xr1, cd, cd + wn,
                                 p0 + 1, p0 + NRC - 1),
                        in_=bass.AP(tensor=x_ap.tensor,
                                    offset=x_ap.offset + (A - 7 + xr0) * W + c0s,
                                    ap=[[A * W, NRC - 2], [W, xr1 - xr0],
                                        [1, wn]]))
                    # rc == 15 (bottom edge: X rows [0, 71))
                    r0, r1 = xr0, min(xr1, XR - 7)
                    nc.sync.dma_start(
                        out=view(OFF_X, r0, r1, cd, cd + wn,
                                 p0 + NRC - 1, p0 + NRC),
                        in_=bass.AP(
                            tensor=x_ap.tensor,
                            offset=x_ap.offset
                            + ((NRC - 1) * A - 7 + r0) * W + c0s,
                            ap=[[0, 1], [W, r1 - r0], [1, wn]]))

            # --------- helpers over the rolling 16-row F/m buffers ---------
            def blk(s):
                return (s % 2) * S          # row block written by slab s

            def fv(base, i, b0, rr0, rr1, x0, x1):
                # rows [b0+rr0, b0+rr1) of dir-i 16-row buffer
                return view(base + i * (SZ_F + PAD),
                            b0 + rr0, b0 + rr1, x0 + 7, x1 + 7)

            bt = big[:, 0:TOTAL]
            BOFF, BPS = bt.offset, bt.ap[0][0]

            def view2(o1, o2, r):
                # 4D [p, 2, rows, cols]: plane t at origin ot; element
                # (row y, col x) of plane t lives at ot + y*PITCH + x
                # (origins fold in the +7 halo shifts).
                a1 = o1 + r[0] * PITCH + r[2]
                a2 = o2 + r[0] * PITCH + r[2]
                return bass.AP(tensor=bt.tensor, offset=BOFF + a1,
                               ap=[[BPS, 128], [a2 - a1, 2],
                                   [PITCH, r[1] - r[0]], [1, r[3] - r[2]]])

            def dbl2(uy, ux, o1, o2, d1, d2, rf):
                # fused x/x^2 8-window sums over rf=(y0,y1,x0,x1) by doubling
                def ext(r, k):
                    return (r[0] + min(k * uy, 0), r[1] + max(k * uy, 0),
                            r[2] + min(k * ux, 0), r[3] + max(k * ux, 0))
                r2 = ext(rf, 4)
                r1 = ext(r2, 2)

                def sh(r, dy, dx):
                    return (r[0] + dy, r[1] + dy, r[2] + dx, r[3] + dx)

                s1o1 = OFF_S1 - r1[0] * PITCH + 7
                s1o2 = OFF_S1B - r1[0] * PITCH + 7
                s2o1 = OFF_S2 - r2[0] * PITCH + 7
                s2o2 = OFF_S2B - r2[0] * PITCH + 7
                nc.vector.tensor_add(view2(s1o1, s1o2, r1),
                                     view2(o1, o2, r1),
                                     view2(o1, o2, sh(r1, uy, ux)))
                nc.vector.tensor_add(view2(s2o1, s2o2, r2),
                                     view2(s1o1, s1o2, r2),
                                     view2(s1o1, s1o2, sh(r2, 2 * uy, 2 * ux)))
                nc.vector.tensor_add(view2(d1, d2, rf),
                                     view2(s2o1, s2o2, rf),
                                     view2(s2o1, s2o2, sh(rf, 4 * uy, 4 * ux)))

            def compute_rows(org, xslot, b0, y0, y1, dirs):
                """fused x1/y2 sums + metric for image rows [y0, y1) of
                `dirs`, into block rows [b0, ...) of the F/m buffers."""
                o1 = OFF_X + 7 * PITCH + 7
                o2 = xslot - org * PITCH + 7
                for j, nm in enumerate(dirs):
                    uy, ux, i, (cx0, cx1) = FWD[nm]
                    rf = (y0, y1, cx0, cx1)
                    nr = y1 - y0
                    y2slot = OFF_Y2 + (j % 2) * (SZ_Y2 + PAD)
                    d1 = OFF_F + i * (SZ_F + PAD) + (b0 - y0) * PITCH + 7
                    d2 = y2slot - y0 * PITCH + 7
                    dbl2(uy, ux, o1, o2, d1, d2, rf)
                    # sq on ACT into the metric buffer, then m = sq/8 - y2
                    mdst = fv(OFF_M, i, b0, 0, nr, cx0, cx1)
                    nc.scalar.square(mdst, fv(OFF_F, i, b0, 0, nr, cx0, cx1))
                    nc.vector.scalar_tensor_tensor(
                        out=mdst, in0=mdst, scalar=0.125,
                        in1=view(y2slot, 0, nr, cx0 + 7, cx1 + 7),
                        op0=mybir.AluOpType.mult,
                        op1=mybir.AluOpType.subtract)

            # ---------------- per-slab compute ----------------
            for s in range(NS):
                s0 = s * S
                xslot = OFF_XSQ + (s % 2) * (SZ_XSQ + PAD)

                if s == 0:
                    org = -7
                    nc.scalar.square(view(xslot, 0, S + 14, 0, PITCH),
                                     xv(-7, S + 7, -7, B + 7))
                    # prologue: rows [-7, 0) into "prev" block rows [S+1,S+8)
                    compute_rows(org, xslot, blk(1) + 1, -7, 0,
                                 ["b", "c", "e"])
                else:
                    org = s0
                    nc.scalar.square(view(xslot, 0, S + 7, 0, PITCH),
                                     xv(s0, s0 + S + 7, -7, B + 7))

                # current slab rows [s0, s0+S) -> block blk(s)
                compute_rows(org, xslot, blk(s), s0, s0 + S, list(FWD))

                # ---------------- select chain ----------------
                bmslot = OFF_BM + (s % 2) * (SZ_O + PAD)
                bxslot = OFF_BX + (s % 2) * (SZ_O + PAD)

                def out_rows(r0, r1):
                    return (view(bmslot, r0, r1, 0, B),
                            view(bxslot, r0, r1, 0, B),
                            lt8[:, r0 * PITCH:r1 * PITCH].rearrange(
                                "p (r c) -> p r c", c=PITCH)[:, :, 0:B])

                def back_segs(nm, dx):
                    # metric/payload rows [s0-7, s0+1), col offset dx:
                    # list of (out_r0, out_r1, block_row0) view segments
                    if s % 2 == 1:
                        return [(0, S, S - 7)]    # prev+cur contiguous
                    if s == 0:
                        # prologue rows [-7,0) live at block rows [S+1,S+8)
                        return [(0, 7, S + 1), (7, S, 0)]
                    return [(0, 7, 2 * S - 7), (7, S, 0)]

                for di, (nm, dy, dx) in enumerate(CHAIN):
                    i = FWD[nm][2]
                    segs = (back_segs(nm, dx) if dy == -7
                            else [(0, S, blk(s))])
                    for (r0, r1, br0) in segs:
                        mv = fv(OFF_M, i, 0, br0, br0 + (r1 - r0), dx, B + dx)
                        xvw = fv(OFF_F, i, 0, br0, br0 + (r1 - r0), dx, B + dx)
                        bmv, bxv, ltv = out_rows(r0, r1)
                        if di == 0:
                            nc.scalar.copy(bmv, mv)
                            nc.scalar.copy(bxv, xvw)
                        else:
                            nc.vector.tensor_tensor(ltv, mv, bmv,
                                                    mybir.AluOpType.is_gt)
                            if di < len(CHAIN) - 1:  # last step: bm unused after
                                nc.vector.copy_predicated(bmv, ltv, mv)
                            nc.vector.copy_predicated(bxv, ltv, xvw)

                # out = bx / 8, split per 32-partition group (wait fan-in)
                oslot = OFF_OUT + (s % 2) * (SZ_O + PAD)
                for g in range(4):
                    nc.scalar.mul(view(oslot, 0, S, 0, B, g * 32, g * 32 + 32),
                                  view(bxslot, 0, S, 0, B, g * 32, g * 32 + 32),
                                  0.125)

                # store: per col-block DMA
                for cb in range(NCB):
                    p0 = cb * NRC
                    nc.sync.dma_start(
                        out=bass.AP(tensor=y_ap.tensor,
                                    offset=y_ap.offset + s0 * W + cb * B,
                                    ap=[[A * W, NRC], [W, S], [1, B]]),
                        in_=view(oslot, 0, S, 0, B, p0, p0 + NRC))
    nc.compile()
    return nc


_nc_cache = []


def _get_nc():
    if not _nc_cache:
        _nc_cache.append(_build())
    return _nc_cache[0]


def kernel(x, weight=None, _want_results=False, **_ignored):
    x = np.ascontiguousarray(np.asarray(x), dtype=np.float32)
    n = x.shape[0]
    assert x.shape == (n, 1, H, W), x.shape
    nc = _get_nc()
    in_maps = [{"x": np.ascontiguousarray(x[i, 0])} for i in range(n)]
    res = run_bass_kernel_spmd(nc, in_maps, core_ids=list(range(n)))
    out = np.stack([r["y"] for r in res.results])[:, None]
    if _want_results:
        return out, res
    return out


if __name__ == "__main__":
    rng = np.random.default_rng(0)
    x = rng.standard_normal((8, 1, H, W)).astype(np.float32)
    y = kernel(x)
    print("ran; out shape", y.shape, "mean", y.mean())



# revision 35
# speedup vs baseline: 1.0096x; 1.0096x over previous
"""Trainium2 Bass kernel for the directional min-variance filter (Kuwahara-style).

Algorithm (per image, fp32):
  For each of 8 directions d (rays of 8 pixels from each pixel):
    x1_d = directional sum of x, y2_d = directional sum of x^2
    metric m'_d = x1_d^2/8 - y2_d          (= y1 - y2; maximize m' == minimize var)
  out = x1_{argmax m'} / 8   with first-index-wins tie semantics (matches argmin).

Layout ("all free-dim"): the 1024x1024 image is split into 128 blocks of
64 rows x 128 cols; partition p = cb*16 + rc owns block (rc, cb) and stores
it row-major with a 7-pixel halo on every side: 78 rows x 142 cols.  Every
directional shift is then a pure free-dim offset, so all sums run on the
vector engine with plain 2-operand adds (log2 doubling: 3 adds per 8-sum).
Forward/backward direction pairs share sums: the backward ray sum at (y,x)
equals the forward sum at (y,x) - 7*u, read as an offset view.

The per-row-slab pipeline (8 rows at a time) keeps forward sums and metrics
in 16-row double-block buffers: each slab computes only its own 8 new rows;
backward-direction views read the previous slab's block (contiguous view for
odd slabs, a 2-way row split for even slabs).

8 NeuronCores run pure data-parallel over the batch of 8 images.
"""

import numpy as np

import concourse.bass as bass
import concourse.bacc as bacc
import concourse.tile as tile
from concourse import mybir
from concourse.bass_utils import run_bass_kernel_spmd

F32 = mybir.dt.float32

H = W = 1024
A = 128         # rows per block
B = 64          # cols per block
NRC = 8         # row-chunks  (rc)
NCB = 16        # col-blocks  (cb)
PITCH = B + 14  # 78
XR = A + 14     # 142 stored rows
S = 16          # slab rows
NS = A // S     # 8 slabs

PAD = 16
SZ_X = XR * PITCH              # 11076
SZ_XSQ = (S + 14) * PITCH      # 30 rows (23 normally; 30 for slab 0)
SZ_S1 = (S + 6) * PITCH
SZ_S2 = (S + 4) * PITCH
SZ_F = 2 * S * PITCH           # two S-row blocks
SZ_Y2 = S * PITCH
SZ_O = S * PITCH

OFF_X = 0
OFF_XSQ = OFF_X + SZ_X + PAD                    # 2 rotating slots
OFF_S1 = OFF_XSQ + 2 * (SZ_XSQ + PAD)           # plane 0 (x) + plane 1 (x^2)
OFF_S1B = OFF_S1 + SZ_S1 + PAD
OFF_S2 = OFF_S1B + SZ_S1 + PAD
OFF_S2B = OFF_S2 + SZ_S2 + PAD
OFF_F = OFF_S2B + SZ_S2 + PAD                   # 4 x 2S-row (x1 per fwd dir)
OFF_Y2 = OFF_F + 4 * (SZ_F + PAD)               # 2 rotating slots
OFF_M = OFF_Y2 + 2 * (SZ_Y2 + PAD)              # 4 x 2S-row (metric per dir)
OFF_BM = OFF_M + 4 * (SZ_F + PAD)               # 2 rotating slots
OFF_BX = OFF_BM + 2 * (SZ_O + PAD)              # 2 rotating slots
OFF_OUT = OFF_BX + 2 * (SZ_O + PAD)             # 2 rotating slots
TOTAL = OFF_OUT + 2 * (SZ_O + PAD) + PITCH

XWAVES = ((0, 46), (46, XR))  # input DMA row waves

# forward dirs: name -> (uy, ux, slot index, F col range (x0, x1))
FWD = {
    "a": (0, 1, 0, (-7, B)),
    "b": (1, 0, 1, (0, B)),
    "c": (1, 1, 2, (-7, B)),
    "e": (1, -1, 3, (0, B + 7)),
}
# chain in reference dir order: (fwd-name, dy, dx): the d-th direction's
# metric/payload = fwd buffer at row offset dy, col offset dx
CHAIN = [
    ("c", -7, -7),  # d0 (-1,-1)
    ("b", -7, 0),   # d1 (-1, 0)
    ("e", -7, 7),   # d2 (-1, 1)
    ("a", 0, -7),   # d3 (0,-1)
    ("a", 0, 0),    # d4 (0, 1)
    ("e", 0, 0),    # d5 (1,-1)
    ("b", 0, 0),    # d6 (1, 0)
    ("c", 0, 0),    # d7 (1, 1)
]


def _build():
    nc = bacc.Bacc("TRN2", target_bir_lowering=False)
    x_t = nc.declare_dram_parameter("x", [H, W], F32, isOutput=False)
    y_t = nc.declare_dram_parameter("y", [H, W], F32, isOutput=True)
    x_ap = x_t[:]
    y_ap = y_t[:]

    with tile.TileContext(nc) as tc:
        with tc.tile_pool(name="main", bufs=1) as pool:
            big = pool.tile([128, TOTAL], F32)
            lt8 = pool.tile([128, S * PITCH + PAD], mybir.dt.uint8)

            def view(off, r0, r1, c0, c1, p0=0, p1=128):
                # [p1-p0, r1-r0, c1-c0] view of a pitch-PITCH buffer at `off`
                start = off + r0 * PITCH + c0
                ln = (r1 - r0) * PITCH
                return big[p0:p1, start:start + ln].rearrange(
                    "p (r c) -> p r c", c=PITCH)[:, :, 0:c1 - c0]

            def xv(y0, y1, x0, x1):
                # X view in image coords (origin row -7, col -7)
                return view(OFF_X, y0 + 7, y1 + 7, x0 + 7, x1 + 7)

            # ------- input load (row-waves of clipped DMAs) -------
            # wave 1 covers X rows [0, 46) so slabs 0-1 can start while
            # wave 2 is still in flight.
            nc.vector.memset(big[:, OFF_X:OFF_X + SZ_X], 0.0)
            for xr0, xr1 in XWAVES:
                for cb in range(NCB):
                    c0s = cb * B - 7
                    wn, cd = PITCH, 0
                    if cb == 0:
                        c0s, wn, cd = 0, PITCH - 7, 7
                    elif cb == NCB - 1:
                        wn = PITCH - 7
                    p0 = cb * NRC
                    # rc == 0 (top edge: img rows [0,71) -> X rows [7,78))
                    r0, r1 = max(xr0, 7), xr1
                    nc.sync.dma_start(
                        out=view(OFF_X, r0, r1, cd, cd + wn, p0, p0 + 1),
                        in_=bass.AP(tensor=x_ap.tensor,
                                    offset=x_ap.offset + (r0 - 7) * W + c0s,
                                    ap=[[0, 1], [W, r1 - r0], [1, wn]]))
                    # rc in [1, 15): X rows [0,78) <- img rows rc*A-7 ...
                    nc.sync.dma_start(
                        out=view(OFF_X, xr0, xr1, cd, cd + wn,
                                 p0 + 1, p0 + NRC - 1),
                        in_=bass.AP(tensor=x_ap.tensor,
                                    offset=x_ap.offset + (A - 7 + xr0) * W + c0s,
                                    ap=[[A * W, NRC - 2], [W, xr1 - xr0],
                                        [1, wn]]))
                    # rc == 15 (bottom edge: X rows [0, 71))
                    r0, r1 = xr0, min(xr1, XR - 7)
                    nc.sync.dma_start(
                        out=view(OFF_X, r0, r1, cd, cd + wn,
                                 p0 + NRC - 1, p0 + NRC),
                        in_=bass.AP(
                            tensor=x_ap.tensor,
                            offset=x_ap.offset
                            + ((NRC - 1) * A - 7 + r0) * W + c0s,
                            ap=[[0, 1], [W, r1 - r0], [1, wn]]))

            # --------- helpers over the rolling 16-row F/m buffers ---------
            def blk(s):
                return (s % 2) * S          # row block written by slab s

            def fv(base, i, b0, rr0, rr1, x0, x1):
                # rows [b0+rr0, b0+rr1) of dir-i 16-row buffer
                return view(base + i * (SZ_F + PAD),
                            b0 + rr0, b0 + rr1, x0 + 7, x1 + 7)

            bt = big[:, 0:TOTAL]
            BOFF, BPS = bt.offset, bt.ap[0][0]

            def view2(o1, o2, r):
                # 4D [p, 2, rows, cols]: plane t at origin ot; element
                # (row y, col x) of plane t lives at ot + y*PITCH + x
                # (origins fold in the +7 halo shifts).
                a1 = o1 + r[0] * PITCH + r[2]
                a2 = o2 + r[0] * PITCH + r[2]
                return bass.AP(tensor=bt.tensor, offset=BOFF + a1,
                               ap=[[BPS, 128], [a2 - a1, 2],
                                   [PITCH, r[1] - r[0]], [1, r[3] - r[2]]])

            def dbl2(uy, ux, o1, o2, d1, d2, rf):
                # fused x/x^2 8-window sums over rf=(y0,y1,x0,x1) by doubling
                def ext(r, k):
                    return (r[0] + min(k * uy, 0), r[1] + max(k * uy, 0),
                            r[2] + min(k * ux, 0), r[3] + max(k * ux, 0))
                r2 = ext(rf, 4)
                r1 = ext(r2, 2)

                def sh(r, dy, dx):
                    return (r[0] + dy, r[1] + dy, r[2] + dx, r[3] + dx)

                s1o1 = OFF_S1 - r1[0] * PITCH + 7
                s1o2 = OFF_S1B - r1[0] * PITCH + 7
                s2o1 = OFF_S2 - r2[0] * PITCH + 7
                s2o2 = OFF_S2B - r2[0] * PITCH + 7
                nc.vector.tensor_add(view2(s1o1, s1o2, r1),
                                     view2(o1, o2, r1),
                                     view2(o1, o2, sh(r1, uy, ux)))
                nc.vector.tensor_add(view2(s2o1, s2o2, r2),
                                     view2(s1o1, s1o2, r2),
                                     view2(s1o1, s1o2, sh(r2, 2 * uy, 2 * ux)))
                nc.vector.tensor_add(view2(d1, d2, rf),
                                     view2(s2o1, s2o2, rf),
                                     view2(s2o1, s2o2, sh(rf, 4 * uy, 4 * ux)))

            def compute_rows(org, xslot, b0, y0, y1, dirs):
                """fused x1/y2 sums + metric for image rows [y0, y1) of
                `dirs`, into block rows [b0, ...) of the F/m buffers."""
                o1 = OFF_X + 7 * PITCH + 7
                o2 = xslot - org * PITCH + 7
                for j, nm in enumerate(dirs):
                    uy, ux, i, (cx0, cx1) = FWD[nm]
                    rf = (y0, y1, cx0, cx1)
                    nr = y1 - y0
                    y2slot = OFF_Y2 + (j % 2) * (SZ_Y2 + PAD)
                    d1 = OFF_F + i * (SZ_F + PAD) + (b0 - y0) * PITCH + 7
                    d2 = y2slot - y0 * PITCH + 7
                    dbl2(uy, ux, o1, o2, d1, d2, rf)
                    # sq on ACT into the metric buffer, then m = sq/8 - y2
                    mdst = fv(OFF_M, i, b0, 0, nr, cx0, cx1)
                    nc.scalar.square(mdst, fv(OFF_F, i, b0, 0, nr, cx0, cx1))
                    nc.vector.scalar_tensor_tensor(
                        out=mdst, in0=mdst, scalar=0.125,
                        in1=view(y2slot, 0, nr, cx0 + 7, cx1 + 7),
                        op0=mybir.AluOpType.mult,
                        op1=mybir.AluOpType.subtract)

            # ---------------- per-slab compute ----------------
            for s in range(NS):
                s0 = s * S
                xslot = OFF_XSQ + (s % 2) * (SZ_XSQ + PAD)

                if s == 0:
                    org = -7
                    nc.scalar.square(view(xslot, 0, S + 14, 0, PITCH),
                                     xv(-7, S + 7, -7, B + 7))
                    # prologue: rows [-7, 0) into "prev" block rows [S+1,S+8)
                    compute_rows(org, xslot, blk(1) + 1, -7, 0,
                                 ["b", "c", "e"])
                else:
                    org = s0
                    nc.scalar.square(view(xslot, 0, S + 7, 0, PITCH),
                                     xv(s0, s0 + S + 7, -7, B + 7))

                # current slab rows [s0, s0+S) -> block blk(s)
                compute_rows(org, xslot, blk(s), s0, s0 + S, list(FWD))

                # ---------------- select chain ----------------
                bmslot = OFF_BM + (s % 2) * (SZ_O + PAD)
                bxslot = OFF_BX + (s % 2) * (SZ_O + PAD)

                def out_rows(r0, r1):
                    return (view(bmslot, r0, r1, 0, B),
                            view(bxslot, r0, r1, 0, B),
                            lt8[:, r0 * PITCH:r1 * PITCH].rearrange(
                                "p (r c) -> p r c", c=PITCH)[:, :, 0:B])

                def back_segs(nm, dx):
                    # metric/payload rows [s0-7, s0+1), col offset dx:
                    # list of (out_r0, out_r1, block_row0) view segments
                    if s % 2 == 1:
                        return [(0, S, S - 7)]    # prev+cur contiguous
                    if s == 0:
                        # prologue rows [-7,0) live at block rows [S+1,S+8)
                        return [(0, 7, S + 1), (7, S, 0)]
                    return [(0, 7, 2 * S - 7), (7, S, 0)]

                for di, (nm, dy, dx) in enumerate(CHAIN):
                    i = FWD[nm][2]
                    segs = (back_segs(nm, dx) if dy == -7
                            else [(0, S, blk(s))])
                    for (r0, r1, br0) in segs:
                        mv = fv(OFF_M, i, 0, br0, br0 + (r1 - r0), dx, B + dx)
                        xvw = fv(OFF_F, i, 0, br0, br0 + (r1 - r0), dx, B + dx)
                        bmv, bxv, ltv = out_rows(r0, r1)
                        if di == 0:
                            nc.scalar.copy(bmv, mv)
                            nc.scalar.copy(bxv, xvw)
                        else:
                            nc.vector.tensor_tensor(ltv, mv, bmv,
                                                    mybir.AluOpType.is_gt)
                            if di < len(CHAIN) - 1:  # last step: bm unused after
                                nc.vector.copy_predicated(bmv, ltv, mv)
                            nc.vector.copy_predicated(bxv, ltv, xvw)

                # out = bx / 8, split per 32-partition group (wait fan-in)
                oslot = OFF_OUT + (s % 2) * (SZ_O + PAD)
                for g in range(4):
                    nc.scalar.mul(view(oslot, 0, S, 0, B, g * 32, g * 32 + 32),
                                  view(bxslot, 0, S, 0, B, g * 32, g * 32 + 32),
                                  0.125)

                # store: per col-block DMA
                for cb in range(NCB):
                    p0 = cb * NRC
                    nc.sync.dma_start(
                        out=bass.AP(tensor=y_ap.tensor,
                                    offset=y_ap.offset + s0 * W + cb * B,
                                    ap=[[A * W, NRC], [W, S], [1, B]]),
                        in_=view(oslot, 0, S, 0, B, p0, p0 + NRC))
    nc.compile()
    return nc


_nc_cache = []


def _get_nc():
    if not _nc_cache:
        _nc_cache.append(_build())
    return _nc_cache[0]


def kernel(x, weight=None, _want_results=False, **_ignored):
    x = np.ascontiguousarray(np.asarray(x), dtype=np.float32)
    n = x.shape[0]
    assert x.shape == (n, 1, H, W), x.shape
    nc = _get_nc()
    in_maps = [{"x": np.ascontiguousarray(x[i, 0])} for i in range(n)]
    res = run_bass_kernel_spmd(nc, in_maps, core_ids=list(range(n)))
    out = np.stack([r["y"] for r in res.results])[:, None]
    if _want_results:
        return out, res
    return out


if __name__ == "__main__":
    rng = np.random.default_rng(0)
    x = rng.standard_normal((8, 1, H, W)).astype(np.float32)
    y = kernel(x)
    print("ran; out shape", y.shape, "mean", y.mean())



# revision 36
# speedup vs baseline: 1.1417x; 1.1308x over previous
"""Trainium2 Bass kernel for the directional min-variance filter (Kuwahara-style).

Algorithm (per image, fp32):
  For each of 8 directions d (rays of 8 pixels from each pixel):
    x1_d = directional sum of x, y2_d = directional sum of x^2
    metric m'_d = x1_d^2/8 - y2_d          (= y1 - y2; maximize m' == minimize var)
  out = x1_{argmax m'} / 8   with first-index-wins tie semantics (matches argmin).

Layout ("all free-dim"): the 1024x1024 image is split into 128 blocks of
64 rows x 128 cols; partition p = cb*16 + rc owns block (rc, cb) and stores
it row-major with a 7-pixel halo on every side: 78 rows x 142 cols.  Every
directional shift is then a pure free-dim offset, so all sums run on the
vector engine with plain 2-operand adds (log2 doubling: 3 adds per 8-sum).
Forward/backward direction pairs share sums: the backward ray sum at (y,x)
equals the forward sum at (y,x) - 7*u, read as an offset view.

The per-row-slab pipeline (8 rows at a time) keeps forward sums and metrics
in 16-row double-block buffers: each slab computes only its own 8 new rows;
backward-direction views read the previous slab's block (contiguous view for
odd slabs, a 2-way row split for even slabs).

8 NeuronCores run pure data-parallel over the batch of 8 images.
"""

import numpy as np

import concourse.bass as bass
import concourse.bacc as bacc
import concourse.tile as tile
from concourse import mybir
from concourse.bass_utils import run_bass_kernel_spmd

F32 = mybir.dt.float32

H = W = 1024
A = 128         # rows per block
B = 64          # cols per block
NRC = 8         # row-chunks  (rc)
NCB = 16        # col-blocks  (cb)
PITCH = B + 14  # 78
XR = A + 14     # 142 stored rows
S = 16          # slab rows
NS = A // S     # 8 slabs

PAD = 16
SZ_X = XR * PITCH              # 11076
SZ_XSQ = (S + 14) * PITCH      # 30 rows (23 normally; 30 for slab 0)
SZ_S1 = (S + 6) * PITCH
SZ_S2 = (S + 4) * PITCH
SZ_F = 2 * S * PITCH           # two S-row blocks
SZ_Y2 = S * PITCH
SZ_O = S * PITCH

OFF_X = 0
OFF_XSQ = OFF_X + SZ_X + PAD                    # 2 rotating slots
OFF_S1 = OFF_XSQ + 2 * (SZ_XSQ + PAD)           # plane 0 (x) + plane 1 (x^2)
OFF_S1B = OFF_S1 + SZ_S1 + PAD
OFF_S2 = OFF_S1B + SZ_S1 + PAD
OFF_S2B = OFF_S2 + SZ_S2 + PAD
OFF_F = OFF_S2B + SZ_S2 + PAD                   # 4 x 2S-row (x1 per fwd dir)
OFF_Y2 = OFF_F + 4 * (SZ_F + PAD)               # 2 rotating slots
OFF_M = OFF_Y2 + 2 * (SZ_Y2 + PAD)              # 4 x 2S-row (metric per dir)
OFF_BM = OFF_M + 4 * (SZ_F + PAD)               # 2 rotating slots
OFF_BX = OFF_BM + 2 * (SZ_O + PAD)              # 2 rotating slots
OFF_OUT = OFF_BX + 2 * (SZ_O + PAD)             # 2 rotating slots
TOTAL = OFF_OUT + 2 * (SZ_O + PAD) + PITCH

XWAVES = ((0, 46), (46, XR))  # input DMA row waves

# forward dirs: name -> (uy, ux, slot index, F col range (x0, x1))
FWD = {
    "a": (0, 1, 0, (-7, B)),
    "b": (1, 0, 1, (0, B)),
    "c": (1, 1, 2, (-7, B)),
    "e": (1, -1, 3, (0, B + 7)),
}
# chain in reference dir order: (fwd-name, dy, dx): the d-th direction's
# metric/payload = fwd buffer at row offset dy, col offset dx
CHAIN = [
    ("c", -7, -7),  # d0 (-1,-1)
    ("b", -7, 0),   # d1 (-1, 0)
    ("e", -7, 7),   # d2 (-1, 1)
    ("a", 0, -7),   # d3 (0,-1)
    ("a", 0, 0),    # d4 (0, 1)
    ("e", 0, 0),    # d5 (1,-1)
    ("b", 0, 0),    # d6 (1, 0)
    ("c", 0, 0),    # d7 (1, 1)
]


def _build():
    nc = bacc.Bacc("TRN2", target_bir_lowering=False)
    x_t = nc.declare_dram_parameter("x", [H, W], F32, isOutput=False)
    y_t = nc.declare_dram_parameter("y", [H, W], F32, isOutput=True)
    x_ap = x_t[:]
    y_ap = y_t[:]

    with tile.TileContext(nc) as tc:
        with tc.tile_pool(name="main", bufs=1) as pool:
            big = pool.tile([128, TOTAL], F32)
            lt8 = pool.tile([128, S * PITCH + PAD], mybir.dt.uint8)

            def view(off, r0, r1, c0, c1, p0=0, p1=128):
                # [p1-p0, r1-r0, c1-c0] view of a pitch-PITCH buffer at `off`
                start = off + r0 * PITCH + c0
                ln = (r1 - r0) * PITCH
                return big[p0:p1, start:start + ln].rearrange(
                    "p (r c) -> p r c", c=PITCH)[:, :, 0:c1 - c0]

            def xv(y0, y1, x0, x1):
                # X view in image coords (origin row -7, col -7)
                return view(OFF_X, y0 + 7, y1 + 7, x0 + 7, x1 + 7)

            # ------- input load (row-waves of clipped DMAs) -------
            # wave 1 covers X rows [0, 46) so slabs 0-1 can start while
            # wave 2 is still in flight.
            nc.vector.memset(big[:, OFF_X:OFF_X + SZ_X], 0.0)
            for xr0, xr1 in XWAVES:
                for cb in range(NCB):
                    c0s = cb * B - 7
                    wn, cd = PITCH, 0
                    if cb == 0:
                        c0s, wn, cd = 0, PITCH - 7, 7
                    elif cb == NCB - 1:
                        wn = PITCH - 7
                    p0 = cb * NRC
                    # rc == 0 (top edge: img rows [0,71) -> X rows [7,78))
                    r0, r1 = max(xr0, 7), xr1
                    nc.sync.dma_start(
                        out=view(OFF_X, r0, r1, cd, cd + wn, p0, p0 + 1),
                        in_=bass.AP(tensor=x_ap.tensor,
                                    offset=x_ap.offset + (r0 - 7) * W + c0s,
                                    ap=[[0, 1], [W, r1 - r0], [1, wn]]))
                    # rc in [1, 15): X rows [0,78) <- img rows rc*A-7 ...
                    nc.sync.dma_start(
                        out=view(OFF_X, xr0, xr1, cd, cd + wn,
                                 p0 + 1, p0 + NRC - 1),
                        in_=bass.AP(tensor=x_ap.tensor,
                                    offset=x_ap.offset + (A - 7 + xr0) * W + c0s,
                                    ap=[[A * W, NRC - 2], [W, xr1 - xr0],
                                        [1, wn]]))
                    # rc == 15 (bottom edge: X rows [0, 71))
                    r0, r1 = xr0, min(xr1, XR - 7)
                    nc.sync.dma_start(
                        out=view(OFF_X, r0, r1, cd, cd + wn,
                                 p0 + NRC - 1, p0 + NRC),
                        in_=bass.AP(
                            tensor=x_ap.tensor,
                            offset=x_ap.offset
                            + ((NRC - 1) * A - 7 + r0) * W + c0s,
                            ap=[[0, 1], [W, r1 - r0], [1, wn]]))

            # --------- helpers over the rolling 16-row F/m buffers ---------
            def blk(s):
                return (s % 2) * S          # row block written by slab s

            def fv(base, i, b0, rr0, rr1, x0, x1):
                # rows [b0+rr0, b0+rr1) of dir-i 16-row buffer
                return view(base + i * (SZ_F + PAD),
                            b0 + rr0, b0 + rr1, x0 + 7, x1 + 7)

            bt = big[:, 0:TOTAL]
            BOFF, BPS = bt.offset, bt.ap[0][0]

            def view2(o1, o2, r):
                # 4D [p, 2, rows, cols]: plane t at origin ot; element
                # (row y, col x) of plane t lives at ot + y*PITCH + x
                # (origins fold in the +7 halo shifts).
                a1 = o1 + r[0] * PITCH + r[2]
                a2 = o2 + r[0] * PITCH + r[2]
                return bass.AP(tensor=bt.tensor, offset=BOFF + a1,
                               ap=[[BPS, 128], [a2 - a1, 2],
                                   [PITCH, r[1] - r[0]], [1, r[3] - r[2]]])

            def dbl2(uy, ux, o1, o2, d1, d2, rf):
                # fused x/x^2 8-window sums over rf=(y0,y1,x0,x1) by doubling
                def ext(r, k):
                    return (r[0] + min(k * uy, 0), r[1] + max(k * uy, 0),
                            r[2] + min(k * ux, 0), r[3] + max(k * ux, 0))
                r2 = ext(rf, 4)
                r1 = ext(r2, 2)

                def sh(r, dy, dx):
                    return (r[0] + dy, r[1] + dy, r[2] + dx, r[3] + dx)

                s1o1 = OFF_S1 - r1[0] * PITCH + 7
                s1o2 = OFF_S1B - r1[0] * PITCH + 7
                s2o1 = OFF_S2 - r2[0] * PITCH + 7
                s2o2 = OFF_S2B - r2[0] * PITCH + 7

                def v3(org, r):
                    # precise 3D view at plane origin `org` (avoids the 4D
                    # range-tracking span across unrelated buffers)
                    a = org + r[0] * PITCH + r[2]
                    return bass.AP(tensor=bt.tensor, offset=BOFF + a,
                                   ap=[[BPS, 128], [PITCH, r[1] - r[0]],
                                       [1, r[3] - r[2]]])

                # L1 separate per plane: keeps the X/XSQ read ranges precise
                # so sums only wait on the DMA rows they actually need.
                nc.vector.tensor_add(v3(s1o1, r1), v3(o1, r1),
                                     v3(o1, sh(r1, uy, ux)))
                nc.vector.tensor_add(v3(s1o2, r1), v3(o2, r1),
                                     v3(o2, sh(r1, uy, ux)))
                nc.vector.tensor_add(view2(s2o1, s2o2, r2),
                                     view2(s1o1, s1o2, r2),
                                     view2(s1o1, s1o2, sh(r2, 2 * uy, 2 * ux)))
                nc.vector.tensor_add(view2(d1, d2, rf),
                                     view2(s2o1, s2o2, rf),
                                     view2(s2o1, s2o2, sh(rf, 4 * uy, 4 * ux)))

            def compute_rows(org, xslot, b0, y0, y1, dirs):
                """fused x1/y2 sums + metric for image rows [y0, y1) of
                `dirs`, into block rows [b0, ...) of the F/m buffers."""
                o1 = OFF_X + 7 * PITCH + 7
                o2 = xslot - org * PITCH + 7
                for j, nm in enumerate(dirs):
                    uy, ux, i, (cx0, cx1) = FWD[nm]
                    rf = (y0, y1, cx0, cx1)
                    nr = y1 - y0
                    y2slot = OFF_Y2 + (j % 2) * (SZ_Y2 + PAD)
                    d1 = OFF_F + i * (SZ_F + PAD) + (b0 - y0) * PITCH + 7
                    d2 = y2slot - y0 * PITCH + 7
                    dbl2(uy, ux, o1, o2, d1, d2, rf)
                    # sq on ACT into the metric buffer, then m = sq/8 - y2
                    mdst = fv(OFF_M, i, b0, 0, nr, cx0, cx1)
                    nc.scalar.square(mdst, fv(OFF_F, i, b0, 0, nr, cx0, cx1))
                    nc.vector.scalar_tensor_tensor(
                        out=mdst, in0=mdst, scalar=0.125,
                        in1=view(y2slot, 0, nr, cx0 + 7, cx1 + 7),
                        op0=mybir.AluOpType.mult,
                        op1=mybir.AluOpType.subtract)

            # ---------------- per-slab compute ----------------
            for s in range(NS):
                s0 = s * S
                xslot = OFF_XSQ + (s % 2) * (SZ_XSQ + PAD)

                if s == 0:
                    org = -7
                    nc.scalar.square(view(xslot, 0, S + 14, 0, PITCH),
                                     xv(-7, S + 7, -7, B + 7))
                    # prologue: rows [-7, 0) into "prev" block rows [S+1,S+8)
                    compute_rows(org, xslot, blk(1) + 1, -7, 0,
                                 ["b", "c", "e"])
                else:
                    org = s0
                    nc.scalar.square(view(xslot, 0, S + 7, 0, PITCH),
                                     xv(s0, s0 + S + 7, -7, B + 7))

                # current slab rows [s0, s0+S) -> block blk(s)
                compute_rows(org, xslot, blk(s), s0, s0 + S, list(FWD))

                # ---------------- select chain ----------------
                bmslot = OFF_BM + (s % 2) * (SZ_O + PAD)
                bxslot = OFF_BX + (s % 2) * (SZ_O + PAD)

                def out_rows(r0, r1):
                    return (view(bmslot, r0, r1, 0, B),
                            view(bxslot, r0, r1, 0, B),
                            lt8[:, r0 * PITCH:r1 * PITCH].rearrange(
                                "p (r c) -> p r c", c=PITCH)[:, :, 0:B])

                def back_segs(nm, dx):
                    # metric/payload rows [s0-7, s0+1), col offset dx:
                    # list of (out_r0, out_r1, block_row0) view segments
                    if s % 2 == 1:
                        return [(0, S, S - 7)]    # prev+cur contiguous
                    if s == 0:
                        # prologue rows [-7,0) live at block rows [S+1,S+8)
                        return [(0, 7, S + 1), (7, S, 0)]
                    return [(0, 7, 2 * S - 7), (7, S, 0)]

                for di, (nm, dy, dx) in enumerate(CHAIN):
                    i = FWD[nm][2]
                    segs = (back_segs(nm, dx) if dy == -7
                            else [(0, S, blk(s))])
                    for (r0, r1, br0) in segs:
                        mv = fv(OFF_M, i, 0, br0, br0 + (r1 - r0), dx, B + dx)
                        xvw = fv(OFF_F, i, 0, br0, br0 + (r1 - r0), dx, B + dx)
                        bmv, bxv, ltv = out_rows(r0, r1)
                        if di == 0:
                            nc.scalar.copy(bmv, mv)
                            nc.scalar.copy(bxv, xvw)
                        else:
                            nc.vector.tensor_tensor(ltv, mv, bmv,
                                                    mybir.AluOpType.is_gt)
                            if di < len(CHAIN) - 1:  # last step: bm unused after
                                nc.vector.copy_predicated(bmv, ltv, mv)
                            nc.vector.copy_predicated(bxv, ltv, xvw)

                # out = bx / 8, split per 32-partition group (wait fan-in)
                oslot = OFF_OUT + (s % 2) * (SZ_O + PAD)
                for g in range(4):
                    nc.scalar.mul(view(oslot, 0, S, 0, B, g * 32, g * 32 + 32),
                                  view(bxslot, 0, S, 0, B, g * 32, g * 32 + 32),
                                  0.125)

                # store: per col-block DMA
                for cb in range(NCB):
                    p0 = cb * NRC
                    nc.sync.dma_start(
                        out=bass.AP(tensor=y_ap.tensor,
                                    offset=y_ap.offset + s0 * W + cb * B,
                                    ap=[[A * W, NRC], [W, S], [1, B]]),
                        in_=view(oslot, 0, S, 0, B, p0, p0 + NRC))
    nc.compile()
    return nc


_nc_cache = []


def _get_nc():
    if not _nc_cache:
        _nc_cache.append(_build())
    return _nc_cache[0]


def kernel(x, weight=None, _want_results=False, **_ignored):
    x = np.ascontiguousarray(np.asarray(x), dtype=np.float32)
    n = x.shape[0]
    assert x.shape == (n, 1, H, W), x.shape
    nc = _get_nc()
    in_maps = [{"x": np.ascontiguousarray(x[i, 0])} for i in range(n)]
    res = run_bass_kernel_spmd(nc, in_maps, core_ids=list(range(n)))
    out = np.stack([r["y"] for r in res.results])[:, None]
    if _want_results:
        return out, res
    return out


if __name__ == "__main__":
    rng = np.random.default_rng(0)
    x = rng.standard_normal((8, 1, H, W)).astype(np.float32)
    y = kernel(x)
    print("ran; out shape", y.shape, "mean", y.mean())



# revision 37
# speedup vs baseline: 1.1577x; 1.0140x over previous
"""Trainium2 Bass kernel for the directional min-variance filter (Kuwahara-style).

Algorithm (per image, fp32):
  For each of 8 directions d (rays of 8 pixels from each pixel):
    x1_d = directional sum of x, y2_d = directional sum of x^2
    metric m'_d = x1_d^2/8 - y2_d          (= y1 - y2; maximize m' == minimize var)
  out = x1_{argmax m'} / 8   with first-index-wins tie semantics (matches argmin).

Layout ("all free-dim"): the 1024x1024 image is split into 128 blocks of
64 rows x 128 cols; partition p = cb*16 + rc owns block (rc, cb) and stores
it row-major with a 7-pixel halo on every side: 78 rows x 142 cols.  Every
directional shift is then a pure free-dim offset, so all sums run on the
vector engine with plain 2-operand adds (log2 doubling: 3 adds per 8-sum).
Forward/backward direction pairs share sums: the backward ray sum at (y,x)
equals the forward sum at (y,x) - 7*u, read as an offset view.

The per-row-slab pipeline (8 rows at a time) keeps forward sums and metrics
in 16-row double-block buffers: each slab computes only its own 8 new rows;
backward-direction views read the previous slab's block (contiguous view for
odd slabs, a 2-way row split for even slabs).

8 NeuronCores run pure data-parallel over the batch of 8 images.
"""

import numpy as np

import concourse.bass as bass
import concourse.bacc as bacc
import concourse.tile as tile
from concourse import mybir
from concourse.bass_utils import run_bass_kernel_spmd

F32 = mybir.dt.float32

H = W = 1024
A = 128         # rows per block
B = 64          # cols per block
NRC = 8         # row-chunks  (rc)
NCB = 16        # col-blocks  (cb)
PITCH = B + 14  # 78
XR = A + 14     # 142 stored rows
S = 16          # slab rows
NS = A // S     # 8 slabs

PAD = 16
SZ_X = XR * PITCH              # 11076
SZ_XSQ = (S + 14) * PITCH      # 30 rows (23 normally; 30 for slab 0)
SZ_S1 = (S + 6) * PITCH
SZ_S2 = (S + 4) * PITCH
SZ_F = 2 * S * PITCH           # two S-row blocks
SZ_Y2 = S * PITCH
SZ_O = S * PITCH

OFF_X = 0
OFF_XSQ = OFF_X + SZ_X + PAD                    # 2 rotating slots
OFF_S1 = OFF_XSQ + 2 * (SZ_XSQ + PAD)           # plane 0 (x) + plane 1 (x^2)
OFF_S1B = OFF_S1 + SZ_S1 + PAD
OFF_S2 = OFF_S1B + SZ_S1 + PAD
OFF_S2B = OFF_S2 + SZ_S2 + PAD
OFF_F = OFF_S2B + SZ_S2 + PAD                   # 4 x 2S-row (x1 per fwd dir)
OFF_Y2 = OFF_F + 4 * (SZ_F + PAD)               # 2 rotating slots
OFF_M = OFF_Y2 + 2 * (SZ_Y2 + PAD)              # 4 x 2S-row (metric per dir)
OFF_BM = OFF_M + 4 * (SZ_F + PAD)               # 2 rotating slots
OFF_BX = OFF_BM + 2 * (SZ_O + PAD)              # 2 rotating slots
OFF_OUT = OFF_BX + 2 * (SZ_O + PAD)             # 2 rotating slots
TOTAL = OFF_OUT + 2 * (SZ_O + PAD) + PITCH

XWAVES = ((0, 46), (46, XR))  # input DMA row waves

# forward dirs: name -> (uy, ux, slot index, F col range (x0, x1))
FWD = {
    "a": (0, 1, 0, (-7, B)),
    "b": (1, 0, 1, (0, B)),
    "c": (1, 1, 2, (-7, B)),
    "e": (1, -1, 3, (0, B + 7)),
}
# chain in reference dir order: (fwd-name, dy, dx): the d-th direction's
# metric/payload = fwd buffer at row offset dy, col offset dx
CHAIN = [
    ("c", -7, -7),  # d0 (-1,-1)
    ("b", -7, 0),   # d1 (-1, 0)
    ("e", -7, 7),   # d2 (-1, 1)
    ("a", 0, -7),   # d3 (0,-1)
    ("a", 0, 0),    # d4 (0, 1)
    ("e", 0, 0),    # d5 (1,-1)
    ("b", 0, 0),    # d6 (1, 0)
    ("c", 0, 0),    # d7 (1, 1)
]


def _build():
    nc = bacc.Bacc("TRN2", target_bir_lowering=False)
    x_t = nc.declare_dram_parameter("x", [H, W], F32, isOutput=False)
    y_t = nc.declare_dram_parameter("y", [H, W], F32, isOutput=True)
    x_ap = x_t[:]
    y_ap = y_t[:]

    with tile.TileContext(nc) as tc:
        with tc.tile_pool(name="main", bufs=1) as pool:
            big = pool.tile([128, TOTAL], F32)
            lt8 = pool.tile([128, S * PITCH + PAD], mybir.dt.uint8)

            def view(off, r0, r1, c0, c1, p0=0, p1=128):
                # [p1-p0, r1-r0, c1-c0] view of a pitch-PITCH buffer at `off`
                start = off + r0 * PITCH + c0
                ln = (r1 - r0) * PITCH
                return big[p0:p1, start:start + ln].rearrange(
                    "p (r c) -> p r c", c=PITCH)[:, :, 0:c1 - c0]

            def xv(y0, y1, x0, x1):
                # X view in image coords (origin row -7, col -7)
                return view(OFF_X, y0 + 7, y1 + 7, x0 + 7, x1 + 7)

            # ------- input load (row-waves of clipped DMAs) -------
            # wave 1 covers X rows [0, 46) so slabs 0-1 can start while
            # wave 2 is still in flight.  Only the halo strips (unwritten on
            # edge partitions) need zeroing; the interior is DMA-overwritten.
            nc.vector.memset(view(OFF_X, 0, 7, 0, PITCH), 0.0)
            nc.vector.memset(view(OFF_X, XR - 7, XR, 0, PITCH), 0.0)
            nc.vector.memset(view(OFF_X, 7, XR - 7, 0, 7), 0.0)
            nc.vector.memset(view(OFF_X, 7, XR - 7, PITCH - 7, PITCH), 0.0)
            for xr0, xr1 in XWAVES:
                for cb in range(NCB):
                    c0s = cb * B - 7
                    wn, cd = PITCH, 0
                    if cb == 0:
                        c0s, wn, cd = 0, PITCH - 7, 7
                    elif cb == NCB - 1:
                        wn = PITCH - 7
                    p0 = cb * NRC
                    # rc == 0 (top edge: img rows [0,71) -> X rows [7,78))
                    r0, r1 = max(xr0, 7), xr1
                    nc.sync.dma_start(
                        out=view(OFF_X, r0, r1, cd, cd + wn, p0, p0 + 1),
                        in_=bass.AP(tensor=x_ap.tensor,
                                    offset=x_ap.offset + (r0 - 7) * W + c0s,
                                    ap=[[0, 1], [W, r1 - r0], [1, wn]]))
                    # rc in [1, 15): X rows [0,78) <- img rows rc*A-7 ...
                    nc.sync.dma_start(
                        out=view(OFF_X, xr0, xr1, cd, cd + wn,
                                 p0 + 1, p0 + NRC - 1),
                        in_=bass.AP(tensor=x_ap.tensor,
                                    offset=x_ap.offset + (A - 7 + xr0) * W + c0s,
                                    ap=[[A * W, NRC - 2], [W, xr1 - xr0],
                                        [1, wn]]))
                    # rc == 15 (bottom edge: X rows [0, 71))
                    r0, r1 = xr0, min(xr1, XR - 7)
                    nc.sync.dma_start(
                        out=view(OFF_X, r0, r1, cd, cd + wn,
                                 p0 + NRC - 1, p0 + NRC),
                        in_=bass.AP(
                            tensor=x_ap.tensor,
                            offset=x_ap.offset
                            + ((NRC - 1) * A - 7 + r0) * W + c0s,
                            ap=[[0, 1], [W, r1 - r0], [1, wn]]))

            # --------- helpers over the rolling 16-row F/m buffers ---------
            def blk(s):
                return (s % 2) * S          # row block written by slab s

            def fv(base, i, b0, rr0, rr1, x0, x1):
                # rows [b0+rr0, b0+rr1) of dir-i 16-row buffer
                return view(base + i * (SZ_F + PAD),
                            b0 + rr0, b0 + rr1, x0 + 7, x1 + 7)

            bt = big[:, 0:TOTAL]
            BOFF, BPS = bt.offset, bt.ap[0][0]

            def view2(o1, o2, r):
                # 4D [p, 2, rows, cols]: plane t at origin ot; element
                # (row y, col x) of plane t lives at ot + y*PITCH + x
                # (origins fold in the +7 halo shifts).
                a1 = o1 + r[0] * PITCH + r[2]
                a2 = o2 + r[0] * PITCH + r[2]
                return bass.AP(tensor=bt.tensor, offset=BOFF + a1,
                               ap=[[BPS, 128], [a2 - a1, 2],
                                   [PITCH, r[1] - r[0]], [1, r[3] - r[2]]])

            def dbl2(uy, ux, o1, o2, d1, d2, rf):
                # fused x/x^2 8-window sums over rf=(y0,y1,x0,x1) by doubling
                def ext(r, k):
                    return (r[0] + min(k * uy, 0), r[1] + max(k * uy, 0),
                            r[2] + min(k * ux, 0), r[3] + max(k * ux, 0))
                r2 = ext(rf, 4)
                r1 = ext(r2, 2)

                def sh(r, dy, dx):
                    return (r[0] + dy, r[1] + dy, r[2] + dx, r[3] + dx)

                s1o1 = OFF_S1 - r1[0] * PITCH + 7
                s1o2 = OFF_S1B - r1[0] * PITCH + 7
                s2o1 = OFF_S2 - r2[0] * PITCH + 7
                s2o2 = OFF_S2B - r2[0] * PITCH + 7

                def v3(org, r):
                    # precise 3D view at plane origin `org` (avoids the 4D
                    # range-tracking span across unrelated buffers)
                    a = org + r[0] * PITCH + r[2]
                    return bass.AP(tensor=bt.tensor, offset=BOFF + a,
                                   ap=[[BPS, 128], [PITCH, r[1] - r[0]],
                                       [1, r[3] - r[2]]])

                # L1 separate per plane: keeps the X/XSQ read ranges precise
                # so sums only wait on the DMA rows they actually need.
                nc.vector.tensor_add(v3(s1o1, r1), v3(o1, r1),
                                     v3(o1, sh(r1, uy, ux)))
                nc.vector.tensor_add(v3(s1o2, r1), v3(o2, r1),
                                     v3(o2, sh(r1, uy, ux)))
                nc.vector.tensor_add(view2(s2o1, s2o2, r2),
                                     view2(s1o1, s1o2, r2),
                                     view2(s1o1, s1o2, sh(r2, 2 * uy, 2 * ux)))
                nc.vector.tensor_add(view2(d1, d2, rf),
                                     view2(s2o1, s2o2, rf),
                                     view2(s2o1, s2o2, sh(rf, 4 * uy, 4 * ux)))

            def compute_rows(org, xslot, b0, y0, y1, dirs):
                """fused x1/y2 sums + metric for image rows [y0, y1) of
                `dirs`, into block rows [b0, ...) of the F/m buffers."""
                o1 = OFF_X + 7 * PITCH + 7
                o2 = xslot - org * PITCH + 7
                for j, nm in enumerate(dirs):
                    uy, ux, i, (cx0, cx1) = FWD[nm]
                    rf = (y0, y1, cx0, cx1)
                    nr = y1 - y0
                    y2slot = OFF_Y2 + (j % 2) * (SZ_Y2 + PAD)
                    d1 = OFF_F + i * (SZ_F + PAD) + (b0 - y0) * PITCH + 7
                    d2 = y2slot - y0 * PITCH + 7
                    dbl2(uy, ux, o1, o2, d1, d2, rf)
                    # sq on ACT into the metric buffer, then m = sq/8 - y2
                    mdst = fv(OFF_M, i, b0, 0, nr, cx0, cx1)
                    nc.scalar.square(mdst, fv(OFF_F, i, b0, 0, nr, cx0, cx1))
                    nc.vector.scalar_tensor_tensor(
                        out=mdst, in0=mdst, scalar=0.125,
                        in1=view(y2slot, 0, nr, cx0 + 7, cx1 + 7),
                        op0=mybir.AluOpType.mult,
                        op1=mybir.AluOpType.subtract)

            # ---------------- per-slab compute ----------------
            for s in range(NS):
                s0 = s * S
                xslot = OFF_XSQ + (s % 2) * (SZ_XSQ + PAD)

                if s == 0:
                    org = -7
                    nc.scalar.square(view(xslot, 0, S + 14, 0, PITCH),
                                     xv(-7, S + 7, -7, B + 7))
                    # prologue: rows [-7, 0) into "prev" block rows [S+1,S+8)
                    compute_rows(org, xslot, blk(1) + 1, -7, 0,
                                 ["b", "c", "e"])
                else:
                    org = s0
                    nc.scalar.square(view(xslot, 0, S + 7, 0, PITCH),
                                     xv(s0, s0 + S + 7, -7, B + 7))

                # current slab rows [s0, s0+S) -> block blk(s)
                compute_rows(org, xslot, blk(s), s0, s0 + S, list(FWD))

                # ---------------- select chain ----------------
                bmslot = OFF_BM + (s % 2) * (SZ_O + PAD)
                bxslot = OFF_BX + (s % 2) * (SZ_O + PAD)

                def out_rows(r0, r1):
                    return (view(bmslot, r0, r1, 0, B),
                            view(bxslot, r0, r1, 0, B),
                            lt8[:, r0 * PITCH:r1 * PITCH].rearrange(
                                "p (r c) -> p r c", c=PITCH)[:, :, 0:B])

                def back_segs(nm, dx):
                    # metric/payload rows [s0-7, s0+1), col offset dx:
                    # list of (out_r0, out_r1, block_row0) view segments
                    if s % 2 == 1:
                        return [(0, S, S - 7)]    # prev+cur contiguous
                    if s == 0:
                        # prologue rows [-7,0) live at block rows [S+1,S+8)
                        return [(0, 7, S + 1), (7, S, 0)]
                    return [(0, 7, 2 * S - 7), (7, S, 0)]

                for di, (nm, dy, dx) in enumerate(CHAIN):
                    i = FWD[nm][2]
                    segs = (back_segs(nm, dx) if dy == -7
                            else [(0, S, blk(s))])
                    for (r0, r1, br0) in segs:
                        mv = fv(OFF_M, i, 0, br0, br0 + (r1 - r0), dx, B + dx)
                        xvw = fv(OFF_F, i, 0, br0, br0 + (r1 - r0), dx, B + dx)
                        bmv, bxv, ltv = out_rows(r0, r1)
                        if di == 0:
                            nc.scalar.copy(bmv, mv)
                            nc.scalar.copy(bxv, xvw)
                        else:
                            nc.vector.tensor_tensor(ltv, mv, bmv,
                                                    mybir.AluOpType.is_gt)
                            if di < len(CHAIN) - 1:  # last step: bm unused after
                                nc.vector.copy_predicated(bmv, ltv, mv)
                            nc.vector.copy_predicated(bxv, ltv, xvw)

                # out = bx / 8, split per 32-partition group (wait fan-in)
                oslot = OFF_OUT + (s % 2) * (SZ_O + PAD)
                for g in range(4):
                    nc.scalar.mul(view(oslot, 0, S, 0, B, g * 32, g * 32 + 32),
                                  view(bxslot, 0, S, 0, B, g * 32, g * 32 + 32),
                                  0.125)

                # store: per col-block DMA
                for cb in range(NCB):
                    p0 = cb * NRC
                    nc.sync.dma_start(
                        out=bass.AP(tensor=y_ap.tensor,
                                    offset=y_ap.offset + s0 * W + cb * B,
                                    ap=[[A * W, NRC], [W, S], [1, B]]),
                        in_=view(oslot, 0, S, 0, B, p0, p0 + NRC))
    nc.compile()
    return nc


_nc_cache = []


def _get_nc():
    if not _nc_cache:
        _nc_cache.append(_build())
    return _nc_cache[0]


def kernel(x, weight=None, _want_results=False, **_ignored):
    x = np.ascontiguousarray(np.asarray(x), dtype=np.float32)
    n = x.shape[0]
    assert x.shape == (n, 1, H, W), x.shape
    nc = _get_nc()
    in_maps = [{"x": np.ascontiguousarray(x[i, 0])} for i in range(n)]
    res = run_bass_kernel_spmd(nc, in_maps, core_ids=list(range(n)))
    out = np.stack([r["y"] for r in res.results])[:, None]
    if _want_results:
        return out, res
    return out


if __name__ == "__main__":
    rng = np.random.default_rng(0)
    x = rng.standard_normal((8, 1, H, W)).astype(np.float32)
    y = kernel(x)
    print("ran; out shape", y.shape, "mean", y.mean())

